# revision 1
# baseline (speedup 1.0000x reference)
"""Trainium2 Bass kernel for windowed multi-head attention.

Shapes (hardcoded): x [1024, 256, 128] fp32, 4 heads x 32 head-dim,
window length N=256. Sharded data-parallel over 8 NeuronCores
(128 windows per core). Weights / bias tables replicated.

Math per window w:
  xe      = x + noise * noise_strength          (host)
  q,k,v   = xe @ Wq*scale, xe @ Wk, xe @ Wv
  S_h     = q_h k_h^T                            [256, 256] per head
  P_h     = exp(S_h) * exp(bias_h)  (bias from rel-pos table; host precomputes exp(bias))
  out_h   = (P_h v_h) / rowsum(P_h)
  y       = concat_h(out_h) @ proj_w + proj_b

On-chip layout: feat-major S^T[m, n] tiles so exp output (P^T) is
directly usable as the stationary operand of the P@v matmuls, which
produce token-major output; softmax denominators come from a ones
column streamed against the same stationary. x^T is produced by the
DMA transpose xbar during the load.
"""

import numpy as np
import ml_dtypes

import concourse.bass as bass
import concourse.tile as tile
from concourse import bacc, mybir
from concourse.bass_utils import run_bass_kernel_spmd

F32 = mybir.dt.float32
BF16 = mybir.dt.bfloat16

N_CORES = 8
B = 1024
N = 256          # tokens per window
DIM = 128
H = 4
HD = 32
WS = 16
BPC = B // N_CORES  # windows per core
SCALE = HD ** -0.5

_cache = {}


def _rel_pos_index():
    coords = np.stack(np.meshgrid(np.arange(WS), np.arange(WS), indexing="ij"))
    cf = coords.reshape(2, -1)
    rc = cf[:, :, None] - cf[:, None, :]
    rc = rc.transpose(1, 2, 0).astype(np.int64)
    rc[..., 0] += WS - 1
    rc[..., 1] += WS - 1
    rc[..., 0] *= 2 * WS - 1
    return rc.sum(-1)  # [N, N]


def build_program(n_windows=BPC, repeat=1):
    nc = bacc.Bacc("TRN2", target_bir_lowering=False, debug=False,
                   num_devices=N_CORES)

    x_d = nc.dram_tensor("x", [n_windows, N, DIM], BF16, kind="ExternalInput").ap()
    # wqp[t] / wkp[t]: columns [w_{2t} | zeros | w_{2t+1} | zeros] so S-matmuls
    # can run K=64 at partition bases {0, 64} (base 96 is illegal on the PE)
    # with the zero rows cancelling the other head's contribution.
    wqp_d = nc.dram_tensor("wqp", [2, DIM, DIM], BF16, kind="ExternalInput").ap()
    wkp_d = nc.dram_tensor("wkp", [2, DIM, DIM], BF16, kind="ExternalInput").ap()
    wv_d = nc.dram_tensor("wv", [DIM, DIM], BF16, kind="ExternalInput").ap()
    pw_d = nc.dram_tensor("pw", [DIM, DIM], BF16, kind="ExternalInput").ap()
    pb_d = nc.dram_tensor("pb", [128, DIM], F32, kind="ExternalInput").ap()
    bias_d = nc.dram_tensor("biasT", [2, 128, 1024], BF16, kind="ExternalInput").ap()
    idb_d = nc.dram_tensor("idb", [128, 128], BF16, kind="ExternalInput").ap()
    y_d = nc.dram_tensor("y", [n_windows, N, DIM], F32, kind="ExternalOutput").ap()

    Exp = mybir.ActivationFunctionType.Exp

    with tile.TileContext(nc) as tc:
        with (
            tc.tile_pool(name="const", bufs=1) as const,
            tc.tile_pool(name="sb", bufs=4) as sb,
            tc.tile_pool(name="ptp", bufs=4) as ptp,
            tc.tile_pool(name="spsum", bufs=2, space="PSUM") as spsum,
            tc.tile_pool(name="mpsum", bufs=4, space="PSUM") as mpsum,
        ):
            wqp = const.tile([128, 256], BF16, tag="wqp")
            nc.sync.dma_start(wqp[:, 0:128], wqp_d[0])
            nc.sync.dma_start(wqp[:, 128:256], wqp_d[1])
            wkp = const.tile([128, 256], BF16, tag="wkp")
            nc.sync.dma_start(wkp[:, 0:128], wkp_d[0])
            nc.sync.dma_start(wkp[:, 128:256], wkp_d[1])
            wv = const.tile([128, 128], BF16, tag="wv")
            nc.sync.dma_start(wv[:], wv_d[:])
            pw = const.tile([128, 128], BF16, tag="pw")
            nc.sync.dma_start(pw[:], pw_d[:])
            pb = const.tile([128, 128], F32, tag="pb")
            nc.sync.dma_start(pb[:], pb_d[:])
            bias0 = const.tile([128, 1024], BF16, tag="bias0")
            nc.sync.dma_start(bias0[:], bias_d[0])
            bias1 = const.tile([128, 1024], BF16, tag="bias1")
            nc.sync.dma_start(bias1[:], bias_d[1])
            idb = const.tile([128, 128], BF16, tag="idb")
            nc.sync.dma_start(idb[:], idb_d[:])
            biases = (bias0, bias1)

            for w in [w for _ in range(repeat) for w in range(n_windows)]:
                # ---- load x^T [c, n] via DMA transpose ----
                xt = sb.tile([128, 256], BF16, tag="xt")
                nc.sync.dma_start(xt[:], x_d[w], transpose=True)

                # ---- q^T, k^T (feat-major, head-pair padded layout) ----
                # rows of pad tile t: [f_{2t}(32) | zeros(32) | f_{2t+1}(32) | zeros(32)]
                qp = mpsum.tile([128, 512], F32, tag="m")
                nc.tensor.matmul(qp[:, 0:256], wqp[:, 0:128], xt[:])
                nc.tensor.matmul(qp[:, 256:512], wqp[:, 128:256], xt[:])
                qps = sb.tile([128, 512], BF16, tag="qps")
                nc.vector.tensor_copy(qps[:], qp[:])
                kp = mpsum.tile([128, 512], F32, tag="m")
                nc.tensor.matmul(kp[:, 0:256], wkp[:, 0:128], xt[:])
                nc.tensor.matmul(kp[:, 256:512], wkp[:, 128:256], xt[:])
                kps = sb.tile([128, 512], BF16, tag="kps")
                nc.vector.tensor_copy(kps[:], kp[:])

                # ---- v (token-major), augmented with a ones column per head
                # so one matmul per (h, mc) yields out_h plus the softmax
                # denominator in the same accumulation group ----
                vp = mpsum.tile([128, 256], F32, tag="m")
                nc.tensor.matmul(vp[:, 0:128], xt[:, 0:128], wv[:])
                nc.tensor.matmul(vp[:, 128:256], xt[:, 128:256], wv[:])
                vs = []
                for mc in range(2):
                    va = sb.tile([128, 132], BF16, tag=f"va{mc}")
                    va3 = va[:].rearrange("p (h c) -> p h c", c=33)
                    vp3 = vp[:, mc * 128:(mc + 1) * 128].rearrange(
                        "p (h c) -> p h c", c=32)
                    nc.vector.tensor_copy(va3[:, :, 0:32], vp3)
                    nc.vector.memset(va3[:, :, 32:33], 1.0)
                    vs.append(va)

                # ---- S^T = (k_h q_h^T) per head, feat-major [m, n] ----
                # tile t holds heads (2t, 2t+1); col = hh*512 + mc*256 + n
                pts = []
                for t in range(2):
                    sp = spsum.tile([128, 1024], F32, tag="s")
                    for hh in range(2):
                        # bias written first (start=True opens the bank's
                        # accumulation group), S-matmuls accumulate onto it
                        nc.tensor.matmul(
                            sp[:, hh * 512:(hh + 1) * 512], idb[:],
                            biases[t][:, hh * 512:(hh + 1) * 512],
                            start=True, stop=False)
                        for mc in range(2):
                            lhs = kps[hh * 64:(hh + 1) * 64,
                                      t * 256 + mc * 128:t * 256 + (mc + 1) * 128]
                            rhs = qps[hh * 64:(hh + 1) * 64, t * 256:(t + 1) * 256]
                            nc.tensor.matmul(
                                sp[:, hh * 512 + mc * 256:hh * 512 + (mc + 1) * 256],
                                lhs, rhs, start=False, stop=(mc == 1))
                    pt = ptp.tile([128, 1024], BF16, tag="pt")
                    nc.scalar.activation(pt[:], sp[:], Exp)
                    pts.append(pt)

                # ---- out_raw = P @ [v|1] accumulated over m chunks ----
                # av cols nc2*132 + h*33 + (0..31) = out_h, +32 = denominator.
                # One matmul per (nc2, h, mc): a single accumulation group is
                # open per PSUM bank at a time (a start=True matmul clears
                # has_written for its whole bank).
                av = mpsum.tile([128, 264], F32, tag="m")
                for nc2 in range(2):
                    for h in range(4):
                        t, hh = divmod(h, 2)
                        for mc in range(2):
                            ps = pts[t][:, hh * 512 + mc * 256 + nc2 * 128:
                                        hh * 512 + mc * 256 + (nc2 + 1) * 128]
                            nc.tensor.matmul(
                                av[:, nc2 * 132 + h * 33:nc2 * 132 + h * 33 + 33],
                                ps, vs[mc][:, h * 33:h * 33 + 33],
                                start=(mc == 0), stop=(mc == 1))

                # ---- normalize, transpose, project ----
                rec = sb.tile([128, 8], F32, tag="rec")
                rec3 = rec[:].rearrange("p (g o) -> p g o", o=1)
                av3 = av[:].rearrange("p (g c) -> p g c", c=33)
                nc.vector.reciprocal(rec3, av3[:, :, 32:33])
                onT = mpsum.tile([128, 256], BF16, tag="m")
                for nc2 in range(2):
                    avh = av[:, nc2 * 132:nc2 * 132 + 132].rearrange(
                        "p (h c) -> p h c", c=33)
                    rech = rec[:, nc2 * 4:(nc2 + 1) * 4].rearrange(
                        "p (h o) -> p h o", o=1)
                    on = sb.tile([128, 128], BF16, tag="on")
                    on3 = on[:].rearrange("p (h c) -> p h c", h=4)
                    nc.vector.tensor_mul(on3, avh[:, :, 0:32],
                                         rech.to_broadcast((128, 4, 32)))
                    nc.tensor.transpose(onT[:, nc2 * 128:(nc2 + 1) * 128],
                                        on[:], idb[:])
                onTs = sb.tile([128, 256], BF16, tag="onTs")
                nc.scalar.copy(onTs[:], onT[:])
                yp = mpsum.tile([128, 256], F32, tag="m")
                for nc2 in range(2):
                    nc.tensor.matmul(yp[:, nc2 * 128:(nc2 + 1) * 128],
                                     onTs[:, nc2 * 128:(nc2 + 1) * 128], pw[:])
                ys = sb.tile([128, 256], F32, tag="ys")
                nc.vector.tensor_add(ys[:, 0:128], yp[:, 0:128], pb[:])
                nc.vector.tensor_add(ys[:, 128:256], yp[:, 128:256], pb[:])
                nc.sync.dma_start(y_d[w, 0:128, :], ys[:, 0:128])
                nc.sync.dma_start(y_d[w, 128:256, :], ys[:, 128:256])

    nc.compile()
    return nc


def host_inputs(x, noise, qkv_w, proj_w, proj_b, bias_table, noise_strength,
                n_windows=BPC, n_cores=N_CORES):
    """Build per-core in_maps from the full-problem inputs."""
    x = np.asarray(x)
    noise = np.asarray(noise)
    qkv_w = np.asarray(qkv_w)
    proj_w = np.asarray(proj_w)
    proj_b = np.asarray(proj_b)
    bias_table = np.asarray(bias_table)
    noise_strength = np.asarray(noise_strength)

    xe = x + noise * noise_strength[0] if noise_strength[0] != 0.0 else x
    xe = np.ascontiguousarray(xe).astype(ml_dtypes.bfloat16)

    wq = (qkv_w[:, 0:DIM] * SCALE).astype(np.float32)
    wk = np.ascontiguousarray(qkv_w[:, DIM:2 * DIM]).astype(np.float32)
    wv = np.ascontiguousarray(qkv_w[:, 2 * DIM:3 * DIM]).astype(ml_dtypes.bfloat16)
    z32 = np.zeros((DIM, 32), np.float32)
    wqp = np.stack([
        np.concatenate([wq[:, 2 * t * 32:(2 * t + 1) * 32], z32,
                        wq[:, (2 * t + 1) * 32:(2 * t + 2) * 32], z32], axis=1)
        for t in range(2)]).astype(ml_dtypes.bfloat16)
    wkp = np.stack([
        np.concatenate([wk[:, 2 * t * 32:(2 * t + 1) * 32], z32,
                        wk[:, (2 * t + 1) * 32:(2 * t + 2) * 32], z32], axis=1)
        for t in range(2)]).astype(ml_dtypes.bfloat16)
    pw = proj_w.astype(ml_dtypes.bfloat16)
    pb = np.broadcast_to(proj_b.astype(np.float32), (128, DIM)).copy()

    # exp(bias) in the S^T tile layout: tile t, partition p=m%128,
    # col hh*512 + mc*256 + n  with h = 2t+hh, m = mc*128+p
    rel = _rel_pos_index()                       # [N, N]
    bias = bias_table[rel.reshape(-1)].reshape(N, N, H).astype(np.float32)
    biasT = np.empty((2, 128, 1024), dtype=np.float32)
    for t in range(2):
        for hh in range(2):
            h = 2 * t + hh
            for mc in range(2):
                blk = bias[:, mc * 128:(mc + 1) * 128, h]  # [n, m_part]
                biasT[t, :, hh * 512 + mc * 256:hh * 512 + (mc + 1) * 256] = blk.T
    biasT = biasT.astype(ml_dtypes.bfloat16)

    idb = np.eye(128, dtype=ml_dtypes.bfloat16)

    shared = dict(wqp=wqp, wkp=wkp, wv=wv, pw=pw, pb=pb, biasT=biasT, idb=idb)
    in_maps = []
    for c in range(n_cores):
        m = dict(shared)
        m["x"] = xe[c * n_windows:(c + 1) * n_windows]
        in_maps.append(m)
    return in_maps


def kernel(**inputs):
    if "nc" not in _cache:
        _cache["nc"] = build_program()
    nc = _cache["nc"]
    in_maps = host_inputs(**inputs)
    res = run_bass_kernel_spmd(nc, in_maps, core_ids=list(range(N_CORES)))
    out = np.concatenate([res.results[c]["y"] for c in range(N_CORES)], axis=0)
    return out



# revision 2
# speedup vs baseline: 2.9399x; 2.9399x over previous
"""Trainium2 Bass kernel for windowed multi-head attention (v2).

Shapes (hardcoded): x [1024, 256, 128] fp32, 4 heads x 32 head-dim,
window length N=256. Sharded data-parallel over 8 NeuronCores
(128 windows per core). Weights / bias tables replicated.

Math per window w:
  q,k,v   = xe @ Wq*scale, xe @ Wk, xe @ Wv        (xe = x host-noised)
  S_h     = q_h k_h^T + bias_h                     [256, 256] per head
  P_h     = exp(S_h)   (either bias seeded in PSUM by PE, or
                        P = exp(S) * expbias on DVE)
  out_h   = (P_h v_h) / rowsum(P_h)
  y       = concat_h(out_h) @ proj_w   (+ proj_b added on host)

v2 changes vs v1:
  - q/k computed unpadded; S matmuls use K=32 at PE tile rows 32h.
  - 8-window batched DMA: one transpose-load and one store per group.
  - y stored bf16 in a [128, BPC*256] feat-contiguous DRAM layout.
  - ones column of the PV augmentation memset once per pool buffer.
  - proj_b folded in on host.
"""

import numpy as np
import ml_dtypes

import concourse.bass as bass
import concourse.tile as tile
from concourse import bacc, mybir

F32 = mybir.dt.float32
BF16 = mybir.dt.bfloat16

N_CORES = 8
B = 1024
N = 256          # tokens per window
DIM = 128
H = 4
HD = 32
WS = 16
BPC = B // N_CORES  # windows per core
GRP = 8             # windows per DMA group
SCALE = HD ** -0.5

BIAS_ON_PE = True    # False: P = exp(S) * expb on DVE
YP_ON_ACT = False     # final PSUM->SBUF output copy on the scalar engine

_cache = {}


def _rel_pos_index():
    coords = np.stack(np.meshgrid(np.arange(WS), np.arange(WS), indexing="ij"))
    cf = coords.reshape(2, -1)
    rc = cf[:, :, None] - cf[:, None, :]
    rc = rc.transpose(1, 2, 0).astype(np.int64)
    rc[..., 0] += WS - 1
    rc[..., 1] += WS - 1
    rc[..., 0] *= 2 * WS - 1
    return rc.sum(-1)  # [N, N]


def build_program(n_windows=BPC, repeat=1):
    assert n_windows % GRP == 0
    n_groups = n_windows // GRP
    nc = bacc.Bacc("TRN2", target_bir_lowering=False, debug=False,
                   num_devices=N_CORES)

    x_d = nc.dram_tensor("x", [n_groups, GRP * N, DIM], BF16,
                         kind="ExternalInput").ap()
    wq_d = nc.dram_tensor("wq", [DIM, DIM], BF16, kind="ExternalInput").ap()
    wk_d = nc.dram_tensor("wk", [DIM, DIM], BF16, kind="ExternalInput").ap()
    wv_d = nc.dram_tensor("wv", [DIM, DIM], BF16, kind="ExternalInput").ap()
    pw_d = nc.dram_tensor("pw", [DIM, DIM], BF16, kind="ExternalInput").ap()
    bias_d = nc.dram_tensor("biasT", [2, 128, 1024], BF16,
                            kind="ExternalInput").ap()
    idb_d = nc.dram_tensor("idb", [128, 128], BF16, kind="ExternalInput").ap()
    y_d = nc.dram_tensor("y", [128, n_windows * N], BF16,
                         kind="ExternalOutput").ap()

    Exp = mybir.ActivationFunctionType.Exp

    with tile.TileContext(nc) as tc:
        with (
            tc.tile_pool(name="const", bufs=1) as const,
            tc.tile_pool(name="xtp", bufs=2) as xtp,
            tc.tile_pool(name="sb", bufs=4) as sb,
            tc.tile_pool(name="vap", bufs=4) as vap,
            tc.tile_pool(name="ptp", bufs=8) as ptp,
            tc.tile_pool(name="ysp", bufs=2) as ysp,
            tc.tile_pool(name="spsum", bufs=2, space="PSUM") as spsum,
            tc.tile_pool(name="qkpp", bufs=1, space="PSUM") as qkpp,
            tc.tile_pool(name="smallp", bufs=3, space="PSUM") as smallp,
        ):
            wq = const.tile([128, 128], BF16, tag="wq")
            nc.sync.dma_start(wq[:], wq_d[:])
            wk = const.tile([128, 128], BF16, tag="wk")
            nc.sync.dma_start(wk[:], wk_d[:])
            wv = const.tile([128, 128], BF16, tag="wv")
            nc.sync.dma_start(wv[:], wv_d[:])
            pw = const.tile([128, 128], BF16, tag="pw")
            nc.sync.dma_start(pw[:], pw_d[:])
            bias0 = const.tile([128, 1024], BF16, tag="bias0")
            nc.sync.dma_start(bias0[:], bias_d[0])
            bias1 = const.tile([128, 1024], BF16, tag="bias1")
            nc.sync.dma_start(bias1[:], bias_d[1])
            idb = const.tile([128, 128], BF16, tag="idb")
            nc.sync.dma_start(idb[:], idb_d[:])
            biases = (bias0, bias1)

            # pre-set the ones column of every va pool buffer and the zero
            # rows of every q3s buffer (never overwritten per-pair)
            va_bufs = []
            for i in range(4):
                va = vap.tile([128, 528], BF16, tag="va")
                va4 = va[:].rearrange("p (b h c) -> p b h c", b=4, c=33)
                nc.vector.memset(va4[:, :, :, 32:33], 1.0)
                va_bufs.append(va)
            q3s_bufs = []
            for i in range(4):
                q3t = vap.tile([128, 512], BF16, tag="q3s")
                nc.vector.memset(q3t[64:96, :], 0.0)
                q3s_bufs.append(q3t)

            pairs = [w for _ in range(repeat) for w in range(0, n_windows, 2)]
            for ji, w0 in enumerate(pairs):
                gw, w8 = divmod(w0, GRP)
                if w8 == 0:
                    # ---- batched transposed load: x^T for GRP windows ----
                    xt8 = xtp.tile([128, GRP * N], BF16, tag="xt")
                    nc.sync.dma_start(xt8[:], x_d[gw], transpose=True)
                    ysb = ysp.tile([128, GRP * N], BF16, tag="ys")
                xtw = [xt8[:, (w8 + w2) * N:(w8 + w2 + 1) * N] for w2 in (0, 1)]

                # ---- q^T, k^T feat-major [4h*32, n] per window ----
                qks_l = []
                for w2 in (0, 1):
                    qkp = qkpp.tile([128, 512], F32, tag="qk")
                    nc.tensor.matmul(qkp[:, 0:256], wq[:], xtw[w2])
                    nc.tensor.matmul(qkp[:, 256:512], wk[:], xtw[w2])
                    qks = sb.tile([128, 512], BF16, tag="qks")
                    nc.vector.tensor_copy(qks[:], qkp[:])
                    qks_l.append(qks)
                # q3 variant [0|q3] rows 64:128 via SBUF copy (rows 64:96
                # pre-zeroed once per buffer); col block w2*256
                q3s = q3s_bufs[ji % 4]
                nc.vector.tensor_copy(
                    q3s[96:128, 0:256], qks_l[0][96:128, 0:256])
                nc.vector.tensor_copy(
                    q3s[96:128, 256:512], qks_l[1][96:128, 0:256])

                # ---- v token-major [m, 4h*32], blocks (mc, w2) ----
                vp = smallp.tile([128, 512], F32, tag="sm")
                for mc in (0, 1):
                    for w2 in (0, 1):
                        nc.tensor.matmul(
                            vp[:, (mc * 2 + w2) * 128:(mc * 2 + w2 + 1) * 128],
                            xtw[w2][:, mc * 128:(mc + 1) * 128], wv[:])
                va = va_bufs[ji % 4]
                va4 = va[:].rearrange("p (b h c) -> p b h c", b=4, c=33)
                vp4 = vp[:].rearrange("p (b h c) -> p b h c", b=4, c=32)
                nc.vector.tensor_copy(va4[:, :, :, 0:32], vp4)

                # ---- S^T pair tiles per head h: [128, 1024], w2-halves ----
                # bias seeds first (idb stationary loaded once); S matmuls
                # rotate PE row groups (h order 0,2,1,3) so each LDWEIGHTS
                # overlaps the previous matmul's stream
                horder = (0, 2, 1, 3)
                sps = {}
                for h in horder:
                    sp = spsum.tile([128, 1024], F32, tag="s")
                    sps[h] = sp
                if BIAS_ON_PE:
                    for h in horder:
                        t, hh = divmod(h, 2)
                        for w2 in (0, 1):
                            nc.tensor.matmul(
                                sps[h][:, w2 * 512:(w2 + 1) * 512], idb[:],
                                biases[t][:, hh * 512:(hh + 1) * 512],
                                start=True, stop=False)
                for w2 in (0, 1):
                    qs = qks_l[w2][:, 0:256]
                    ks = qks_l[w2][:, 256:512]
                    for mc in range(2):
                        for h in horder:
                            if h < 3:
                                lhs = ks[32 * h:32 * (h + 1),
                                         mc * 128:(mc + 1) * 128]
                                rhs = qs[32 * h:32 * (h + 1), :]
                            else:
                                # K=64 at base 64: zero rows 64:96 of the
                                # q3 variant cancel k2
                                lhs = ks[64:128, mc * 128:(mc + 1) * 128]
                                rhs = q3s[64:128, w2 * 256:(w2 + 1) * 256]
                            nc.tensor.matmul(
                                sps[h][:, w2 * 512 + mc * 256:
                                       w2 * 512 + (mc + 1) * 256],
                                lhs, rhs,
                                start=(not BIAS_ON_PE),
                                stop=(mc == 1) if BIAS_ON_PE else True)
                pts = {}
                for h in horder:
                    t, hh = divmod(h, 2)
                    pt = ptp.tile([128, 1024], BF16, tag="pt")
                    nc.scalar.activation(pt[:], sps[h][:], Exp)
                    if not BIAS_ON_PE:
                        ptb = ptp.tile([128, 1024], BF16, tag="ptb")
                        bb = biases[t][:, hh * 512:(hh + 1) * 512]
                        bb2 = bb.rearrange("p n -> p 1 n").to_broadcast(
                            (128, 2, 512))
                        pt3 = pt[:].rearrange("p (a n) -> p a n", a=2)
                        ptb3 = ptb[:].rearrange("p (a n) -> p a n", a=2)
                        nc.vector.tensor_mul(ptb3, pt3, bb2)
                        pt = ptb
                    pts[h] = pt

                # ---- out_raw = P @ [v|1] accumulated over m chunks ----
                avs = []
                for w2 in (0, 1):
                    av = smallp.tile([128, 264], F32, tag="sm")
                    for nc2 in range(2):
                        for h in range(4):
                            for mc in range(2):
                                ps = pts[h][:, w2 * 512 + mc * 256 + nc2 * 128:
                                            w2 * 512 + mc * 256 + (nc2 + 1) * 128]
                                nc.tensor.matmul(
                                    av[:, nc2 * 132 + h * 33:
                                       nc2 * 132 + h * 33 + 33],
                                    ps,
                                    va[:, (mc * 2 + w2) * 132 + h * 33:
                                       (mc * 2 + w2) * 132 + h * 33 + 33],
                                    start=(mc == 0), stop=(mc == 1))
                    avs.append(av)

                # ---- normalize, transpose, project (pair-fused) ----
                on = sb.tile([128, 512], BF16, tag="on")
                for w2 in (0, 1):
                    av = avs[w2]
                    rec = sb.tile([128, 8], F32, tag="rec")
                    rec3 = rec[:].rearrange("p (g o) -> p g o", o=1)
                    av3 = av[:].rearrange("p (g c) -> p g c", c=33)
                    nc.vector.reciprocal(rec3, av3[:, :, 32:33])
                    onw = on[:, w2 * 256:(w2 + 1) * 256]
                    on4 = onw.rearrange("p (m h c) -> p m h c", m=2, c=32)
                    av4 = av[:].rearrange("p (m h c) -> p m h c", m=2, c=33)
                    rec4 = rec[:].rearrange("p (m h o) -> p m h o", m=2, o=1)
                    nc.vector.tensor_mul(on4, av4[:, :, :, 0:32],
                                         rec4.to_broadcast((128, 2, 4, 32)))
                onT = smallp.tile([128, 512], BF16, tag="sm")
                for w2 in (0, 1):
                    for nc2 in range(2):
                        nc.tensor.transpose(
                            onT[:, (w2 * 2 + nc2) * 128:(w2 * 2 + nc2 + 1) * 128],
                            on[:, w2 * 256 + nc2 * 128:w2 * 256 + (nc2 + 1) * 128],
                            idb[:])
                onTs = sb.tile([128, 512], BF16, tag="onTs")
                nc.vector.tensor_copy(onTs[:], onT[:])
                # y^T[dout, (w2 nc2 n)] = pw^T @ out^T: constant stationary,
                # one matmul for the whole pair; y stays feature-major
                yp = smallp.tile([128, 512], F32, tag="sm")
                nc.tensor.matmul(yp[:], pw[:], onTs[:])
                if YP_ON_ACT:
                    nc.scalar.copy(ysb[:, w8 * N:(w8 + 2) * N], yp[:])
                else:
                    nc.vector.tensor_copy(ysb[:, w8 * N:(w8 + 2) * N], yp[:])

                if w8 == GRP - 2:
                    nc.sync.dma_start(
                        y_d[:, (w0 - GRP + 2) * N:(w0 + 2) * N], ysb[:])

    nc.compile()
    return nc


def host_inputs(x, noise, qkv_w, proj_w, proj_b, bias_table, noise_strength,
                n_windows=BPC, n_cores=N_CORES):
    """Build per-core in_maps from the full-problem inputs."""
    x = np.asarray(x)
    noise = np.asarray(noise)
    qkv_w = np.asarray(qkv_w)
    proj_w = np.asarray(proj_w)
    bias_table = np.asarray(bias_table)
    noise_strength = np.asarray(noise_strength)

    xe = x + noise * noise_strength[0] if noise_strength[0] != 0.0 else x
    xe = np.ascontiguousarray(xe).astype(ml_dtypes.bfloat16)

    wq = (qkv_w[:, 0:DIM] * SCALE).astype(ml_dtypes.bfloat16)
    wk = np.ascontiguousarray(qkv_w[:, DIM:2 * DIM]).astype(ml_dtypes.bfloat16)
    wv = np.ascontiguousarray(qkv_w[:, 2 * DIM:3 * DIM]).astype(ml_dtypes.bfloat16)
    pw = proj_w.astype(ml_dtypes.bfloat16)

    # bias in the S^T tile layout: tile t, partition p=m%128,
    # col hh*512 + mc*256 + n  with h = 2t+hh, m = mc*128+p
    rel = _rel_pos_index()                       # [N, N]
    bias = bias_table[rel.reshape(-1)].reshape(N, N, H).astype(np.float32)
    biasT = np.empty((2, 128, 1024), dtype=np.float32)
    for t in range(2):
        for hh in range(2):
            h = 2 * t + hh
            for mc in range(2):
                blk = bias[:, mc * 128:(mc + 1) * 128, h]  # [n, m_part]
                biasT[t, :, hh * 512 + mc * 256:hh * 512 + (mc + 1) * 256] = blk.T
    if not BIAS_ON_PE:
        biasT = np.exp(biasT)
    biasT = biasT.astype(ml_dtypes.bfloat16)

    idb = np.eye(128, dtype=ml_dtypes.bfloat16)

    shared = dict(wq=wq, wk=wk, wv=wv, pw=pw, biasT=biasT, idb=idb)
    in_maps = []
    for c in range(n_cores):
        m = dict(shared)
        m["x"] = xe[c * n_windows:(c + 1) * n_windows].reshape(
            n_windows // GRP, GRP * N, DIM)
        in_maps.append(m)
    return in_maps


def unshard_output(results, proj_b, n_windows=BPC, n_cores=N_CORES):
    """results: list of per-core {"y": [128, n_windows*N] bf16} maps."""
    outs = []
    for c in range(n_cores):
        # y is stored feature-major: y_d[dout, w*256 + nc2*128 + n]
        yc = np.asarray(results[c]["y"]).astype(np.float32)  # [128, nw*256]
        yc = yc.reshape(128, n_windows, 2, 128)
        yc = yc.transpose(1, 2, 3, 0).reshape(n_windows, N, DIM)
        outs.append(yc)
    y = np.concatenate(outs, axis=0)
    pb = np.asarray(proj_b, np.float32)
    if np.any(pb != 0.0):
        y = y + pb
    return y


def kernel(**inputs):
    from concourse.bass_utils import run_bass_kernel_spmd
    if "nc" not in _cache:
        _cache["nc"] = build_program()
    nc = _cache["nc"]
    in_maps = host_inputs(**inputs)
    res = run_bass_kernel_spmd(nc, in_maps, core_ids=list(range(N_CORES)))
    return unshard_output(res.results, inputs["proj_b"])
